# revision 4
# baseline (speedup 1.0000x reference)
"""Jones congruence kernel (V_p = J1 @ V_m @ J2^T per baseline/time/freq) on 8 trn2 cores.

Sharding: time axis (64) split across 8 cores (8 t-steps each); every core
runs an identical program on its time slice.

v2: fp16 end-to-end on device. V is cast to fp16 on the host (same precision
loss as the v1 on-device cast), O is produced fp16 and upcast host-side —
this halves HBM traffic (134MB -> 68MB per core). The elementwise stages run
as fused multi-plane DVE ops using stride-0 broadcast APs (full 2x fp16 rate,
one instruction overhead per 4 planes), with the final stage-2 adds offloaded
to GpSimd so DVE, GpSimd and ACT loads are balanced:

Per-core pipeline, 16 groups of 126 baselines:
  - V streams in as fp16 via HWDGE (2MB/group, prefetched 2 ahead).
  - j1/j2 are antenna gathers done as TensorEngine one-hot matmuls:
    onehot[128ant, 126bl]^T @ jones[128ant, sites] -> PSUM (f32), then
    ScalarE copies PSUM->SBUF casting to fp16 (512-site chunks, 2 PSUM bufs).
  - stage 1 (DVE): m1[a,b,c] = j1[a,b] * V[b,c] as 2 fused 4-plane muls
    (j1 operand broadcast over c, V over a); T[a,c] = m1[a,0,c] + m1[a,1,c]
    as 1 fused 4-plane add.
  - stage 2: m2[a,d,c] = T[a,c] * j2[d,c] as 2 fused 4-plane muls (DVE);
    O[a,d] = m2[a,d,0] + m2[a,d,1] as 4 per-plane adds on GpSimd.
  - O streams out as fp16 via HWDGE.
"""
import sys
sys.path.insert(0, "/opt/trn_rl_repo")
import numpy as np

NPOL, NANT, NBL, NTIMES, NFREQS = 2, 64, 2016, 64, 256
N_CORES = 8
T_LOC = NTIMES // N_CORES          # 8 timesteps per core
PLANE = T_LOC * NFREQS             # 2048 sites per (q, baseline) plane
GROUP = 126                        # baselines per tile group
N_GROUPS = NBL // GROUP            # 16
S_CHUNK = 512                      # psum gather chunk (one bank per q plane)
N_CHUNKS = PLANE // S_CHUNK        # 4

_cache = {}


def _split_excess_waits(nc, mybir):
    """Walrus in this env rejects >2 sem-wait conditions per instruction.
    Insert Drain clones carrying the excess waits immediately before."""
    fn = nc.m.functions[0]

    def walk(blocks):
        for bb in blocks:
            yield bb
            yield from walk(getattr(bb, "blocks", None) or [])

    ctr = [0]
    for bb in walk(fn.blocks):
        newlist = []
        for ins in bb.instructions:
            si = ins.sync_info
            if si is not None and si.on_wait and len(si.on_wait) > 1:
                waits = list(si.on_wait)
                while len(waits) > 1:
                    chunk, waits = waits[:1], waits[1:]
                    d = mybir.InstNoOp(
                        name=f"waitsplit-{ctr[0]}",
                        engine=ins.engine,
                        ins=[],
                        outs=[],
                        sync_info=mybir.SyncInfo(on_wait=chunk, on_update=[]),
                    )
                    ctr[0] += 1
                    newlist.append(d)
                si.on_wait = waits
            newlist.append(ins)
        bb.instructions = newlist


def _build():
    import concourse.bass as bass
    import concourse.tile as tile
    from concourse import mybir
    from contextlib import ExitStack

    f32, f16 = mybir.dt.float32, mybir.dt.float16
    nc = bass.Bass("TRN2", target_bir_lowering=False, debug=False,
                   dynamic_dma_scratch_size=2048)
    V = nc.dram_tensor("V", [NBL, 4, PLANE], f16, kind="ExternalInput").ap()
    J = nc.dram_tensor("J", [NANT, 4, PLANE], f16, kind="ExternalInput").ap()
    W = nc.dram_tensor("W", [128, 2 * NBL], f16, kind="ExternalInput").ap()
    O = nc.dram_tensor("O", [NBL, 4, PLANE], f16, kind="ExternalOutput").ap()

    with tile.TileContext(nc) as tc:
        with ExitStack() as ctx:
            fixp = ctx.enter_context(tc.tile_pool(name="fix", bufs=1))
            viop = ctx.enter_context(tc.tile_pool(name="vio", bufs=2))
            outp = ctx.enter_context(tc.tile_pool(name="out", bufs=2))
            jp = ctx.enter_context(tc.tile_pool(name="jp", bufs=2))
            tp = ctx.enter_context(tc.tile_pool(name="tp", bufs=1))
            m1p = ctx.enter_context(tc.tile_pool(name="m1p", bufs=1))
            pp = ctx.enter_context(tc.tile_pool(name="pp", bufs=2, space="PSUM"))

            # jones moving table [128, 4, 2048] fp16 (rows 64-127 zero)
            jt = fixp.tile([128, 4, PLANE], f16, name="jt")
            nc.sync.dma_start(jt[:NANT], J[:])
            nc.vector.memset(jt[NANT:], 0.0)

            # software-pipelined V + one-hot-weight prefetch, depth 2
            vtiles = {}
            wtiles = {}

            def vin(g):
                n0 = g * GROUP
                vtiles[g] = viop.tile([GROUP, 4, PLANE], f16, tag="v",
                                      name=f"v_{g}")
                nc.sync.dma_start(vtiles[g][:], V[n0:n0 + GROUP])
                w1t = jp.tile([128, GROUP], f16, tag="w1", name=f"w1_{g}")
                w2t = jp.tile([128, GROUP], f16, tag="w2", name=f"w2_{g}")
                nc.sync.dma_start(w1t[:], W[:, n0:n0 + GROUP])
                nc.sync.dma_start(w2t[:], W[:, NBL + n0:NBL + n0 + GROUP])
                wtiles[g] = (w1t, w2t)

            vin(0)
            vin(1)

            for g in range(N_GROUPS):
                n0 = g * GROUP
                v = vtiles.pop(g)
                ot = outp.tile([GROUP, 4, PLANE], f16, tag="ot", name=f"ot_{g}")

                # --- gather j1/j2 via one-hot matmuls, S_CHUNK sites/bank ---
                j1s = jp.tile([GROUP, 4, PLANE], f16, tag="j1", name="j1s")
                j2s = jp.tile([GROUP, 4, PLANE], f16, tag="j2", name="j2s")
                w1, w2 = wtiles.pop(g)

                def gather(js, w):
                    for c in range(N_CHUNKS):
                        s0 = c * S_CHUNK
                        pj = pp.tile([GROUP, 4, S_CHUNK], f32, tag="pj", name="pj")
                        for q in range(4):
                            nc.tensor.matmul(pj[:, q, :], w, jt[:, q, s0:s0 + S_CHUNK])
                        nc.scalar.copy(js[:, :, s0:s0 + S_CHUNK], pj[:])

                gather(j1s, w1[:])
                gather(j2s, w2[:])

                # --- stage 1 (DVE): m1[a,b,c] = j1[a,b] (.) V[b,c] -------
                # fused per-a: out [p, b, c, s]; j1 bcast over c, V natural.
                m1 = m1p.tile([GROUP, 8, PLANE], f16, tag="m1", name="m1")
                vbc = v[:].rearrange("p (b c) s -> p b c s", b=2, c=2)
                for a in (0, 1):
                    j1b = j1s[:, 2 * a:2 * a + 2, :].unsqueeze(2).broadcast_to(
                        [GROUP, 2, 2, PLANE])
                    nc.vector.tensor_mul(
                        m1[:, 4 * a:4 * a + 4, :].rearrange(
                            "p (b c) s -> p b c s", b=2, c=2),
                        j1b, vbc)
                # T[a,c] = m1[a,0,c] + m1[a,1,c]  (1 fused 4-plane add)
                T = tp.tile([GROUP, 4, PLANE], f16, tag="T", name="T")
                m1v = m1[:].rearrange("p (a b c) s -> p a b c s", a=2, b=2, c=2)
                nc.vector.tensor_add(
                    T[:].rearrange("p (a c) s -> p a c s", a=2, c=2),
                    m1v[:, :, 0, :, :], m1v[:, :, 1, :, :])

                # --- stage 2 (DVE muls, GpSimd adds) ----------------------
                # m2[a,d,c] = T[a,c] (.) j2[d,c]; fused per-a: T bcast over d.
                # Products overwrite dead buffers: a=0 -> j1s, a=1 -> v
                # (both fully consumed by the stage-1 muls; same-engine order
                # makes the WAR dependency free).
                j2dc = j2s[:].rearrange("p (d c) s -> p d c s", d=2, c=2)
                m2dst = {0: j1s, 1: v}
                for a in (0, 1):
                    tb = T[:, 2 * a:2 * a + 2, :].unsqueeze(1).broadcast_to(
                        [GROUP, 2, 2, PLANE])
                    nc.vector.tensor_mul(
                        m2dst[a][:].rearrange("p (d c) s -> p d c s", d=2, c=2),
                        tb, j2dc)
                # O[a,d] = m2[a,d,0] + m2[a,d,1]  (4 per-plane adds, GpSimd)
                for a in (0, 1):
                    for dpol in (0, 1):
                        nc.gpsimd.tensor_add(ot[:, 2 * a + dpol, :],
                                             m2dst[a][:, 2 * dpol, :],
                                             m2dst[a][:, 2 * dpol + 1, :])

                if g + 2 < N_GROUPS:
                    vin(g + 2)
                if g == N_GROUPS - 1:
                    # shorten the tail: ship each plane as soon as it is added
                    nc.sync.dma_start(O[n0:n0 + GROUP, 0:1], ot[:, 0:1, :])
                    nc.sync.dma_start(O[n0:n0 + GROUP, 1:2], ot[:, 1:2, :])
                    nc.sync.dma_start(O[n0:n0 + GROUP, 2:3], ot[:, 2:3, :])
                    nc.sync.dma_start(O[n0:n0 + GROUP, 3:4], ot[:, 3:4, :])
                else:
                    nc.sync.dma_start(O[n0:n0 + GROUP], ot[:])

    _split_excess_waits(nc, mybir)
    return nc


def _prep_inputs(V_m, jones, ant1, ant2):
    """Per-core input maps: V/J time-sliced fp16, one-hot weights replicated."""
    wt = np.zeros((128, 2 * NBL), dtype=np.float16)
    wt[ant1, np.arange(NBL)] = 1.0
    wt[ant2, NBL + np.arange(NBL)] = 1.0
    in_maps = []
    for k in range(N_CORES):
        t0 = k * T_LOC
        vk = np.ascontiguousarray(
            V_m[:, :, :, t0:t0 + T_LOC, :].transpose(2, 0, 1, 3, 4)
        ).astype(np.float16).reshape(NBL, 4, PLANE)
        jk = np.ascontiguousarray(
            jones[:, :, :, t0:t0 + T_LOC, :].transpose(2, 0, 1, 3, 4)
        ).astype(np.float16).reshape(NANT, 4, PLANE)
        in_maps.append({"V": vk, "J": jk, "W": wt})
    return in_maps


def kernel(V_m, jones, ant1, ant2):
    from concourse.bass_utils import run_bass_kernel_spmd

    V_m = np.asarray(V_m, dtype=np.float32)
    jones = np.asarray(jones, dtype=np.float32)
    a1 = np.asarray(ant1).astype(np.int64)
    a2 = np.asarray(ant2).astype(np.int64)

    if "nc" not in _cache:
        _cache["nc"] = _build()
    nc = _cache["nc"]

    in_maps = _prep_inputs(V_m, jones, a1, a2)
    res = run_bass_kernel_spmd(nc, in_maps, list(range(N_CORES)))
    out = np.empty((NPOL, NPOL, NBL, NTIMES, NFREQS), dtype=np.float32)
    for k in range(N_CORES):
        t0 = k * T_LOC
        out[:, :, :, t0:t0 + T_LOC, :] = res.results[k]["O"].astype(
            np.float32).reshape(
            NBL, NPOL, NPOL, T_LOC, NFREQS).transpose(1, 2, 0, 3, 4)
    return out


# revision 13
# speedup vs baseline: 180.8186x; 180.8186x over previous
"""Jones congruence kernel (V_p = J1 @ V_m @ J2^T per baseline/time/freq) on 8 trn2 cores.

Sharding: time axis (64) split across 8 cores (8 t-steps each); every core
runs an identical program on its time slice. Measured (neuron-profile NTFF)
device exec time ~462us/core, vs ~704us for the f32-I/O per-plane-op
baseline; rel err 7.6e-4 (threshold 2e-2).

fp16 end-to-end on device: V is cast to fp16 on the host (same precision
loss as an on-device cast, but half the HBM read traffic), O is produced
fp16 and upcast host-side — HBM traffic per core is 68MB vs 134MB for f32
I/O. The elementwise work runs entirely on DVE as four fused multi-plane
tensor_tensor ops per group, using stride-0 broadcast APs; these run at the
full 2x_1p fp16 rate (~1.106us per 126x2048 plane, 24 planes/group), which
HW-measures as the binding resource (DVE ~92% busy). Everything else
(PE gathers, ACT PSUM copies, DMA) fits underneath.

Engine notes baked into this design (all HW-measured here):
  - DVE fused broadcast ops hold the 2x rate; op overhead ~150ns.
  - A concurrent GpSimd tensor op halves DVE throughput (port interaction),
    and GpSimd itself runs ~5-8us/plane — so GpSimd does only the startup
    memset and is otherwise idle.
  - ACT ACTIVATE costs ~2us per 2048-elem/lane op regardless of source, so
    the PSUM->SBUF gather copies stay in 512-site x 4-plane chunks (8/group
    = ~16us, under the DVE 26.5us/group).
  - scalar_tensor_tensor has NO DVE perf modes (would be 2x slower than
    separate mul+add); tensor_scalar's 4x modes need per-partition scalars.

Per-core pipeline, 16 groups of 126 baselines:
  - V streams in as fp16 via HWDGE (2MB/group, prefetched 2 ahead); the
    tiny one-hot W slices are issued first so gathers unblock early.
  - j1/j2 are antenna gathers done as TensorEngine one-hot matmuls:
    onehot[128ant, 126bl]^T @ jones[128ant, sites] -> PSUM (f32), then
    ScalarE copies PSUM->SBUF casting to fp16 (512-site chunks, 2 PSUM bufs).
  - stage 1 (DVE): m1[a,b,c] = j1[a,b] * V[b,c] as one fused 8-plane mul
    (j1 broadcast over c, V over a); T[a,c] = m1[a,0,c] + m1[a,1,c] as one
    fused 4-plane add.
  - stage 2 (DVE): m2[a,d,c] = T[a,c] * j2[d,c] as one fused 8-plane mul
    written into the (dead) m1 scratch (same-engine WAR is free);
    O[a,d] = m2[a,d,0] + m2[a,d,1] as one fused 4-plane add.
  - O streams out as fp16 via HWDGE; the last group ships per-plane to
    shorten the tail.
"""
import sys
sys.path.insert(0, "/opt/trn_rl_repo")
import numpy as np

NPOL, NANT, NBL, NTIMES, NFREQS = 2, 64, 2016, 64, 256
N_CORES = 8
T_LOC = NTIMES // N_CORES          # 8 timesteps per core
PLANE = T_LOC * NFREQS             # 2048 sites per (q, baseline) plane
GROUP = 126                        # baselines per tile group
N_GROUPS = NBL // GROUP            # 16
S_CHUNK = 512                      # psum gather chunk (one bank per q plane)
N_CHUNKS = PLANE // S_CHUNK        # 4

_cache = {}


def _split_excess_waits(nc, mybir):
    """Walrus in this env rejects >2 sem-wait conditions per instruction.
    Insert Drain clones carrying the excess waits immediately before."""
    fn = nc.m.functions[0]

    def walk(blocks):
        for bb in blocks:
            yield bb
            yield from walk(getattr(bb, "blocks", None) or [])

    ctr = [0]
    for bb in walk(fn.blocks):
        newlist = []
        for ins in bb.instructions:
            si = ins.sync_info
            if si is not None and si.on_wait and len(si.on_wait) > 1:
                waits = list(si.on_wait)
                while len(waits) > 1:
                    chunk, waits = waits[:1], waits[1:]
                    d = mybir.InstNoOp(
                        name=f"waitsplit-{ctr[0]}",
                        engine=ins.engine,
                        ins=[],
                        outs=[],
                        sync_info=mybir.SyncInfo(on_wait=chunk, on_update=[]),
                    )
                    ctr[0] += 1
                    newlist.append(d)
                si.on_wait = waits
            newlist.append(ins)
        bb.instructions = newlist


def _build():
    import concourse.bass as bass
    import concourse.tile as tile
    from concourse import mybir
    from contextlib import ExitStack

    f32, f16 = mybir.dt.float32, mybir.dt.float16
    nc = bass.Bass("TRN2", target_bir_lowering=False, debug=False,
                   dynamic_dma_scratch_size=2048)
    V = nc.dram_tensor("V", [NBL, 4, PLANE], f16, kind="ExternalInput").ap()
    J = nc.dram_tensor("J", [NANT, 4, PLANE], f16, kind="ExternalInput").ap()
    W = nc.dram_tensor("W", [128, 2 * NBL], f16, kind="ExternalInput").ap()
    O = nc.dram_tensor("O", [NBL, 4, PLANE], f16, kind="ExternalOutput").ap()

    with tile.TileContext(nc) as tc:
        with ExitStack() as ctx:
            fixp = ctx.enter_context(tc.tile_pool(name="fix", bufs=1))
            viop = ctx.enter_context(tc.tile_pool(name="vio", bufs=2))
            outp = ctx.enter_context(tc.tile_pool(name="out", bufs=2))
            jp = ctx.enter_context(tc.tile_pool(name="jp", bufs=2))
            tp = ctx.enter_context(tc.tile_pool(name="tp", bufs=1))
            m1p = ctx.enter_context(tc.tile_pool(name="m1p", bufs=1))
            pp = ctx.enter_context(tc.tile_pool(name="pp", bufs=2, space="PSUM"))

            # jones moving table [128, 4, 2048] fp16 (rows 64-127 zero)
            jt = fixp.tile([128, 4, PLANE], f16, name="jt")
            nc.sync.dma_start(jt[:NANT], J[:])
            # zero the tail rows on the (otherwise idle) GpSimd so the DVE
            # pipeline isn't gated by a 7us memset at startup
            nc.gpsimd.memset(jt[NANT:], 0.0)

            # software-pipelined V + one-hot-weight prefetch, depth 2
            vtiles = {}
            wtiles = {}

            def vin(g):
                n0 = g * GROUP
                # w first: the tiny one-hot slices gate the gather matmuls,
                # the big V transfer only gates the later stage-1 muls
                w1t = jp.tile([128, GROUP], f16, tag="w1", name=f"w1_{g}")
                w2t = jp.tile([128, GROUP], f16, tag="w2", name=f"w2_{g}")
                nc.sync.dma_start(w1t[:], W[:, n0:n0 + GROUP])
                nc.sync.dma_start(w2t[:], W[:, NBL + n0:NBL + n0 + GROUP])
                wtiles[g] = (w1t, w2t)
                vtiles[g] = viop.tile([GROUP, 4, PLANE], f16, tag="v",
                                      name=f"v_{g}")
                nc.sync.dma_start(vtiles[g][:], V[n0:n0 + GROUP])

            vin(0)
            vin(1)

            for g in range(N_GROUPS):
                n0 = g * GROUP
                v = vtiles.pop(g)
                ot = outp.tile([GROUP, 4, PLANE], f16, tag="ot", name=f"ot_{g}")

                # --- gather j1/j2 via one-hot matmuls, S_CHUNK sites/bank ---
                j1s = jp.tile([GROUP, 4, PLANE], f16, tag="j1", name="j1s")
                j2s = jp.tile([GROUP, 4, PLANE], f16, tag="j2", name="j2s")
                w1, w2 = wtiles.pop(g)

                def gather(js, w):
                    for c in range(N_CHUNKS):
                        s0 = c * S_CHUNK
                        pj = pp.tile([GROUP, 4, S_CHUNK], f32, tag="pj", name="pj")
                        for q in range(4):
                            nc.tensor.matmul(pj[:, q, :], w, jt[:, q, s0:s0 + S_CHUNK])
                        nc.scalar.copy(js[:, :, s0:s0 + S_CHUNK], pj[:])

                gather(j1s, w1[:])
                gather(j2s, w2[:])

                # --- stage 1 (DVE): m1[a,b,c] = j1[a,b] (.) V[b,c] -------
                # one fused 8-plane mul: j1 bcast over c, V bcast over a.
                m1 = m1p.tile([GROUP, 8, PLANE], f16, tag="m1", name="m1")
                j1b = j1s[:].rearrange("p (a b) s -> p a b s", a=2, b=2
                                       ).unsqueeze(3).broadcast_to(
                    [GROUP, 2, 2, 2, PLANE])
                vbc = v[:].rearrange("p (b c) s -> p b c s", b=2, c=2
                                     ).unsqueeze(1).broadcast_to(
                    [GROUP, 2, 2, 2, PLANE])
                nc.vector.tensor_mul(
                    m1[:].rearrange("p (a b c) s -> p a b c s", a=2, b=2, c=2),
                    j1b, vbc)
                # T[a,c] = m1[a,0,c] + m1[a,1,c]  (1 fused 4-plane add)
                T = tp.tile([GROUP, 4, PLANE], f16, tag="T", name="T")
                m1v = m1[:].rearrange("p (a b c) s -> p a b c s", a=2, b=2, c=2)
                nc.vector.tensor_add(
                    T[:].rearrange("p (a c) s -> p a c s", a=2, c=2),
                    m1v[:, :, 0, :, :], m1v[:, :, 1, :, :])

                # --- stage 2 (DVE muls + split add) -----------------------
                # m2[a,d,c] = T[a,c] (.) j2[d,c] as one fused 8-plane mul.
                # Products overwrite the dead m1 scratch (fully consumed by
                # the stage-1 add; same-engine order makes the WAR free).
                tb = T[:].rearrange("p (a c) s -> p a c s", a=2, c=2
                                    ).unsqueeze(2).broadcast_to(
                    [GROUP, 2, 2, 2, PLANE])
                j2b = j2s[:].rearrange("p (d c) s -> p d c s", d=2, c=2
                                       ).unsqueeze(1).broadcast_to(
                    [GROUP, 2, 2, 2, PLANE])
                nc.vector.tensor_mul(
                    m1[:].rearrange("p (a d c) s -> p a d c s", a=2, d=2, c=2),
                    tb, j2b)
                # O[a,d] = m2[a,d,0] + m2[a,d,1]  (1 fused 4-plane add).
                # All-DVE on purpose: a concurrent GpSimd tensor op halves
                # DVE throughput (measured), so GpSimd must stay quiet.
                m2v = m1[:].rearrange("p (a d c) s -> p a d c s", a=2, d=2, c=2)
                nc.vector.tensor_add(
                    ot[:].rearrange("p (a d) s -> p a d s", a=2, d=2),
                    m2v[:, :, :, 0, :], m2v[:, :, :, 1, :])

                if g + 2 < N_GROUPS:
                    vin(g + 2)
                if g == N_GROUPS - 1:
                    # shorten the tail: ship each plane as soon as it is added
                    nc.sync.dma_start(O[n0:n0 + GROUP, 0:1], ot[:, 0:1, :])
                    nc.sync.dma_start(O[n0:n0 + GROUP, 1:2], ot[:, 1:2, :])
                    nc.sync.dma_start(O[n0:n0 + GROUP, 2:3], ot[:, 2:3, :])
                    nc.sync.dma_start(O[n0:n0 + GROUP, 3:4], ot[:, 3:4, :])
                else:
                    nc.sync.dma_start(O[n0:n0 + GROUP], ot[:])

    _split_excess_waits(nc, mybir)
    return nc


def _prep_inputs(V_m, jones, ant1, ant2):
    """Per-core input maps: V/J time-sliced fp16, one-hot weights replicated."""
    wt = np.zeros((128, 2 * NBL), dtype=np.float16)
    wt[ant1, np.arange(NBL)] = 1.0
    wt[ant2, NBL + np.arange(NBL)] = 1.0
    in_maps = []
    for k in range(N_CORES):
        t0 = k * T_LOC
        vk = np.ascontiguousarray(
            V_m[:, :, :, t0:t0 + T_LOC, :].transpose(2, 0, 1, 3, 4)
        ).astype(np.float16).reshape(NBL, 4, PLANE)
        jk = np.ascontiguousarray(
            jones[:, :, :, t0:t0 + T_LOC, :].transpose(2, 0, 1, 3, 4)
        ).astype(np.float16).reshape(NANT, 4, PLANE)
        in_maps.append({"V": vk, "J": jk, "W": wt})
    return in_maps


def kernel(V_m, jones, ant1, ant2):
    from concourse.bass_utils import run_bass_kernel_spmd

    V_m = np.asarray(V_m, dtype=np.float32)
    jones = np.asarray(jones, dtype=np.float32)
    a1 = np.asarray(ant1).astype(np.int64)
    a2 = np.asarray(ant2).astype(np.int64)

    if "nc" not in _cache:
        _cache["nc"] = _build()
    nc = _cache["nc"]

    in_maps = _prep_inputs(V_m, jones, a1, a2)
    res = run_bass_kernel_spmd(nc, in_maps, list(range(N_CORES)))
    out = np.empty((NPOL, NPOL, NBL, NTIMES, NFREQS), dtype=np.float32)
    for k in range(N_CORES):
        t0 = k * T_LOC
        out[:, :, :, t0:t0 + T_LOC, :] = res.results[k]["O"].astype(
            np.float32).reshape(
            NBL, NPOL, NPOL, T_LOC, NFREQS).transpose(1, 2, 0, 3, 4)
    return out


# revision 20
# speedup vs baseline: 182.0006x; 1.0065x over previous
"""Jones congruence kernel (V_p = J1 @ V_m @ J2^T per baseline/time/freq) on 8 trn2 cores.

Sharding: time axis (64) split across 8 cores (8 t-steps each); every core
runs an identical program on its time slice. Measured (neuron-profile NTFF)
device exec time ~459us/core, vs ~704us for the f32-I/O per-plane-op
baseline; rel err 7.6e-4 (threshold 2e-2). DVE is ~92% busy at its 2x_1p
rate — the remaining ~35us is pipeline ramp (V arrival + first gather)
and drain.

fp16 end-to-end on device: V is cast to fp16 on the host (same precision
loss as an on-device cast, but half the HBM read traffic), O is produced
fp16 and upcast host-side — HBM traffic per core is 68MB vs 134MB for f32
I/O. The elementwise work runs entirely on DVE as four fused multi-plane
tensor_tensor ops per group, using stride-0 broadcast APs; these run at the
full 2x_1p fp16 rate (~1.106us per 126x2048 plane, 24 planes/group), which
HW-measures as the binding resource (DVE ~92% busy). Everything else
(PE gathers, ACT PSUM copies, DMA) fits underneath.

Engine notes baked into this design (all HW-measured here):
  - DVE fused broadcast ops hold the 2x rate; op overhead ~150ns.
  - A concurrent GpSimd tensor op halves DVE throughput (port interaction),
    and GpSimd itself runs ~5-8us/plane — so GpSimd does only the startup
    memset and is otherwise idle.
  - ACT ACTIVATE costs ~2us per 2048-elem/lane op regardless of source, so
    the PSUM->SBUF gather copies stay in 512-site x 4-plane chunks (8/group
    = ~16us, under the DVE 26.5us/group).
  - scalar_tensor_tensor has NO DVE perf modes (would be 2x slower than
    separate mul+add); tensor_scalar's 4x modes need per-partition scalars.

Per-core pipeline, 16 groups of 126 baselines:
  - V streams in as fp16 via HWDGE (2MB/group, prefetched 2 ahead); the
    tiny one-hot W slices are issued first so gathers unblock early.
  - j1/j2 are antenna gathers done as TensorEngine one-hot matmuls over
    K=64 antenna rows (no zero-padding to 128, no startup memset; the
    jones table loads per-plane so the first matmul starts ~1us in):
    onehot[64ant, 126bl]^T @ jones[64ant, sites] -> PSUM (f32), then
    ScalarE copies PSUM->SBUF casting to fp16 (512-site chunks, 2 PSUM bufs).
  - stage 1 (DVE): m1[a,b,c] = j1[a,b] * V[b,c] as one fused 8-plane mul
    (j1 broadcast over c, V over a); T[a,c] = m1[a,0,c] + m1[a,1,c] as one
    fused 4-plane add.
  - stage 2 (DVE): m2[a,d,c] = T[a,c] * j2[d,c] as one fused 8-plane mul
    written into the (dead) m1 scratch (same-engine WAR is free);
    O[a,d] = m2[a,d,0] + m2[a,d,1] as one fused 4-plane add.
  - O streams out as fp16 via HWDGE; the last group ships per-plane to
    shorten the tail.
"""
import sys
sys.path.insert(0, "/opt/trn_rl_repo")
import numpy as np

NPOL, NANT, NBL, NTIMES, NFREQS = 2, 64, 2016, 64, 256
N_CORES = 8
T_LOC = NTIMES // N_CORES          # 8 timesteps per core
PLANE = T_LOC * NFREQS             # 2048 sites per (q, baseline) plane
GROUP = 126                        # baselines per tile group
N_GROUPS = NBL // GROUP            # 16
S_CHUNK = 512                      # psum gather chunk (one bank per q plane)
N_CHUNKS = PLANE // S_CHUNK        # 4

_cache = {}


def _split_excess_waits(nc, mybir):
    """Walrus in this env rejects >2 sem-wait conditions per instruction.
    Insert Drain clones carrying the excess waits immediately before."""
    fn = nc.m.functions[0]

    def walk(blocks):
        for bb in blocks:
            yield bb
            yield from walk(getattr(bb, "blocks", None) or [])

    ctr = [0]
    for bb in walk(fn.blocks):
        newlist = []
        for ins in bb.instructions:
            si = ins.sync_info
            if si is not None and si.on_wait and len(si.on_wait) > 1:
                waits = list(si.on_wait)
                while len(waits) > 1:
                    chunk, waits = waits[:1], waits[1:]
                    d = mybir.InstNoOp(
                        name=f"waitsplit-{ctr[0]}",
                        engine=ins.engine,
                        ins=[],
                        outs=[],
                        sync_info=mybir.SyncInfo(on_wait=chunk, on_update=[]),
                    )
                    ctr[0] += 1
                    newlist.append(d)
                si.on_wait = waits
            newlist.append(ins)
        bb.instructions = newlist


def _build():
    import concourse.bass as bass
    import concourse.tile as tile
    from concourse import mybir
    from contextlib import ExitStack

    f32, f16 = mybir.dt.float32, mybir.dt.float16
    nc = bass.Bass("TRN2", target_bir_lowering=False, debug=False,
                   dynamic_dma_scratch_size=2048)
    V = nc.dram_tensor("V", [NBL, 4, PLANE], f16, kind="ExternalInput").ap()
    J = nc.dram_tensor("J", [NANT, 4, PLANE], f16, kind="ExternalInput").ap()
    W = nc.dram_tensor("W", [NANT, 2 * NBL], f16, kind="ExternalInput").ap()
    O = nc.dram_tensor("O", [NBL, 4, PLANE], f16, kind="ExternalOutput").ap()

    with tile.TileContext(nc) as tc:
        with ExitStack() as ctx:
            fixp = ctx.enter_context(tc.tile_pool(name="fix", bufs=1))
            viop = ctx.enter_context(tc.tile_pool(name="vio", bufs=2))
            outp = ctx.enter_context(tc.tile_pool(name="out", bufs=2))
            jp = ctx.enter_context(tc.tile_pool(name="jp", bufs=2))
            tp = ctx.enter_context(tc.tile_pool(name="tp", bufs=1))
            m1p = ctx.enter_context(tc.tile_pool(name="m1p", bufs=1))
            pp = ctx.enter_context(tc.tile_pool(name="pp", bufs=2, space="PSUM"))

            # jones moving table [64, 4, 2048] fp16. The gather matmuls
            # contract over K=64 antenna rows only — no zero-padded tail,
            # no startup memset, and per-plane loads unblock the first
            # gather matmul ~1us after launch.
            jt = fixp.tile([NANT, 4, PLANE], f16, name="jt")
            for q in range(4):
                nc.sync.dma_start(jt[:, q], J[:, q])

            # software-pipelined V + one-hot-weight prefetch, depth 2
            vtiles = {}
            wtiles = {}

            def vin(g):
                n0 = g * GROUP
                # w first: the tiny one-hot slices gate the gather matmuls,
                # the big V transfer only gates the later stage-1 muls
                w1t = jp.tile([NANT, GROUP], f16, tag="w1", name=f"w1_{g}")
                w2t = jp.tile([NANT, GROUP], f16, tag="w2", name=f"w2_{g}")
                nc.sync.dma_start(w1t[:], W[:, n0:n0 + GROUP])
                nc.sync.dma_start(w2t[:], W[:, NBL + n0:NBL + n0 + GROUP])
                wtiles[g] = (w1t, w2t)
                vtiles[g] = viop.tile([GROUP, 4, PLANE], f16, tag="v",
                                      name=f"v_{g}")
                nc.sync.dma_start(vtiles[g][:], V[n0:n0 + GROUP])

            vin(0)
            vin(1)

            for g in range(N_GROUPS):
                n0 = g * GROUP
                v = vtiles.pop(g)
                ot = outp.tile([GROUP, 4, PLANE], f16, tag="ot", name=f"ot_{g}")

                # --- gather j1/j2 via one-hot matmuls, S_CHUNK sites/bank ---
                j1s = jp.tile([GROUP, 4, PLANE], f16, tag="j1", name="j1s")
                j2s = jp.tile([GROUP, 4, PLANE], f16, tag="j2", name="j2s")
                w1, w2 = wtiles.pop(g)

                def gather(js, w):
                    for c in range(N_CHUNKS):
                        s0 = c * S_CHUNK
                        pj = pp.tile([GROUP, 4, S_CHUNK], f32, tag="pj", name="pj")
                        for q in range(4):
                            nc.tensor.matmul(pj[:, q, :], w, jt[:, q, s0:s0 + S_CHUNK])
                        nc.scalar.copy(js[:, :, s0:s0 + S_CHUNK], pj[:])

                gather(j1s, w1[:])
                gather(j2s, w2[:])

                # --- stage 1 (DVE): m1[a,b,c] = j1[a,b] (.) V[b,c] -------
                # one fused 8-plane mul: j1 bcast over c, V bcast over a.
                m1 = m1p.tile([GROUP, 8, PLANE], f16, tag="m1", name="m1")
                j1b = j1s[:].rearrange("p (a b) s -> p a b s", a=2, b=2
                                       ).unsqueeze(3).broadcast_to(
                    [GROUP, 2, 2, 2, PLANE])
                vbc = v[:].rearrange("p (b c) s -> p b c s", b=2, c=2
                                     ).unsqueeze(1).broadcast_to(
                    [GROUP, 2, 2, 2, PLANE])
                nc.vector.tensor_mul(
                    m1[:].rearrange("p (a b c) s -> p a b c s", a=2, b=2, c=2),
                    j1b, vbc)
                # T[a,c] = m1[a,0,c] + m1[a,1,c]  (1 fused 4-plane add)
                T = tp.tile([GROUP, 4, PLANE], f16, tag="T", name="T")
                m1v = m1[:].rearrange("p (a b c) s -> p a b c s", a=2, b=2, c=2)
                nc.vector.tensor_add(
                    T[:].rearrange("p (a c) s -> p a c s", a=2, c=2),
                    m1v[:, :, 0, :, :], m1v[:, :, 1, :, :])

                # --- stage 2 (DVE muls + split add) -----------------------
                # m2[a,d,c] = T[a,c] (.) j2[d,c] as one fused 8-plane mul.
                # Products overwrite the dead m1 scratch (fully consumed by
                # the stage-1 add; same-engine order makes the WAR free).
                tb = T[:].rearrange("p (a c) s -> p a c s", a=2, c=2
                                    ).unsqueeze(2).broadcast_to(
                    [GROUP, 2, 2, 2, PLANE])
                j2b = j2s[:].rearrange("p (d c) s -> p d c s", d=2, c=2
                                       ).unsqueeze(1).broadcast_to(
                    [GROUP, 2, 2, 2, PLANE])
                nc.vector.tensor_mul(
                    m1[:].rearrange("p (a d c) s -> p a d c s", a=2, d=2, c=2),
                    tb, j2b)
                # O[a,d] = m2[a,d,0] + m2[a,d,1]  (fused add on DVE).
                # All-DVE on purpose: a concurrent GpSimd tensor op halves
                # DVE throughput (measured), so GpSimd must stay quiet.
                m2v = m1[:].rearrange("p (a d c) s -> p a d c s", a=2, d=2, c=2)
                if g == N_GROUPS - 1:
                    # tail: add + ship per a-pair so the last DMA drain
                    # overlaps the final add
                    for a in (0, 1):
                        nc.vector.tensor_add(
                            ot[:, 2 * a:2 * a + 2, :].unsqueeze(1),
                            m2v[:, a:a + 1, :, 0, :], m2v[:, a:a + 1, :, 1, :])
                        nc.sync.dma_start(O[n0:n0 + GROUP, 2 * a:2 * a + 2],
                                          ot[:, 2 * a:2 * a + 2, :])
                else:
                    nc.vector.tensor_add(
                        ot[:].rearrange("p (a d) s -> p a d s", a=2, d=2),
                        m2v[:, :, :, 0, :], m2v[:, :, :, 1, :])
                    vin(g + 2) if g + 2 < N_GROUPS else None
                    nc.sync.dma_start(O[n0:n0 + GROUP], ot[:])

    _split_excess_waits(nc, mybir)
    return nc


def _prep_inputs(V_m, jones, ant1, ant2):
    """Per-core input maps: V/J time-sliced fp16, one-hot weights replicated."""
    wt = np.zeros((NANT, 2 * NBL), dtype=np.float16)
    wt[ant1, np.arange(NBL)] = 1.0
    wt[ant2, NBL + np.arange(NBL)] = 1.0
    in_maps = []
    for k in range(N_CORES):
        t0 = k * T_LOC
        vk = np.ascontiguousarray(
            V_m[:, :, :, t0:t0 + T_LOC, :].transpose(2, 0, 1, 3, 4)
        ).astype(np.float16).reshape(NBL, 4, PLANE)
        jk = np.ascontiguousarray(
            jones[:, :, :, t0:t0 + T_LOC, :].transpose(2, 0, 1, 3, 4)
        ).astype(np.float16).reshape(NANT, 4, PLANE)
        in_maps.append({"V": vk, "J": jk, "W": wt})
    return in_maps


def kernel(V_m, jones, ant1, ant2):
    from concourse.bass_utils import run_bass_kernel_spmd

    V_m = np.asarray(V_m, dtype=np.float32)
    jones = np.asarray(jones, dtype=np.float32)
    a1 = np.asarray(ant1).astype(np.int64)
    a2 = np.asarray(ant2).astype(np.int64)

    if "nc" not in _cache:
        _cache["nc"] = _build()
    nc = _cache["nc"]

    in_maps = _prep_inputs(V_m, jones, a1, a2)
    res = run_bass_kernel_spmd(nc, in_maps, list(range(N_CORES)))
    out = np.empty((NPOL, NPOL, NBL, NTIMES, NFREQS), dtype=np.float32)
    for k in range(N_CORES):
        t0 = k * T_LOC
        out[:, :, :, t0:t0 + T_LOC, :] = res.results[k]["O"].astype(
            np.float32).reshape(
            NBL, NPOL, NPOL, T_LOC, NFREQS).transpose(1, 2, 0, 3, 4)
    return out
